# revision 26
# baseline (speedup 1.0000x reference)
"""Trainium2 Bass kernel for nn_LowpassDetector (4th-order Butterworth IIR
lowpass over [T=65536, C=512], zero initial conditions).

Approach: the filter's slowest pole has |p| = 0.7577, so the impulse
response decays below fp32 resolution after ~100 taps (|h[128]| ~ 2e-16).
The sequential IIR is therefore numerically identical (at fp32) to a
128-tap causal FIR.  A 128-sample time block then satisfies

    y_blk[n] = A @ x_blk[n] + B @ x_blk[n-1]

with A[i,j] = h[i-j] (lower-triangular) and B[i,j] = h[i+128-j]
(strictly upper-triangular), which maps onto two TensorEngine matmuls
per block accumulating in one PSUM bank.

Sharding: time axis across the 8 cores (8192 steps each) with a
128-sample halo block from the previous shard (zeros for core 0);
channels (512) ride the matmul free dimension.  Host prepends the halo,
gathers per-core outputs.
"""

import os
from contextlib import ExitStack

import numpy as np

import concourse.bass as bass
import concourse.mybir as mybir
import concourse.tile as tile
from concourse import bacc
from concourse._compat import get_trn_type
from concourse.bass_utils import run_bass_kernel_spmd

T, C = 65536, 512
NCORES = 8
TL = T // NCORES            # 8192 timesteps per core
B = 128                     # time block (partition dim / conv matrix size)
NBLK = TL // B              # 64 output blocks per core
SUP = 4                     # blocks per superblock DMA (1 MB transfers)
NSUP = NBLK // SUP          # 16 output superblocks per core
IN_ROWS = TL + B            # 8320 input rows per core (halo + shard)
C2 = 2 * C                  # fp16 hi|lo interleaved row width

ORDER = 4
CUTOFF = 20e9
SAMPLERATE = 160e9
RESPONSIVITY = 1.0
F32 = mybir.dt.float32


def _butter_lowpass(order, wn):
    """Digital Butterworth lowpass (b, a); same math as the model."""
    fs = 2.0
    warped = 2.0 * fs * np.tan(np.pi * wn / fs)
    m = np.arange(-order + 1, order, 2)
    p = -np.exp(1j * np.pi * m / (2.0 * order))
    p = warped * p
    k = warped**order
    fs2 = 2.0 * fs
    pz = (fs2 + p) / (fs2 - p)
    zz = -np.ones(order)
    kz = k * np.real(1.0 / np.prod(fs2 - p))
    b = np.real(kz * np.poly(zz))
    a = np.real(np.poly(pz))
    return b, a


def _impulse_response():
    b, a = _butter_lowpass(ORDER, 2.0 * CUTOFF / SAMPLERATE)
    # impulse response in float64 via the DFII-T recurrence
    K = 2 * B
    h = np.zeros(K)
    z = np.zeros(ORDER)
    for n in range(K):
        xn = 1.0 if n == 0 else 0.0
        y = b[0] * xn + z[0]
        z = np.concatenate([z[1:], [0.0]]) + b[1:] * xn - a[1:] * y
        h[n] = y
    return h * RESPONSIVITY


def _conv_mats():
    """Block-convolution matrices (float64, untransposed)."""
    h = _impulse_response()
    K = 2 * B
    i = np.arange(B)[:, None]
    j = np.arange(B)[None, :]
    A = np.where(i >= j, h[np.clip(i - j, 0, K - 1)], 0.0)
    Bm = h[i + B - j]  # i + B - j in [1, 2B-1]; h[k] ~ 0 for k >= 128 anyway
    return A, Bm


def _split16(m):
    """Split float64 matrix into fp16 hi + fp16 lo (hi + lo ~= m)."""
    hi = m.astype(np.float16)
    lo = (m - hi.astype(np.float64)).astype(np.float16)
    return hi, lo


def _conv_mats_fp16():
    """lhsT weight matrices for the matmuls: transposed fp16 hi/lo splits."""
    A, Bm = _conv_mats()
    ah, al = _split16(A.T)
    bh, bl = _split16(Bm.T)
    return (
        np.ascontiguousarray(ah),
        np.ascontiguousarray(al),
        np.ascontiguousarray(bh),
        np.ascontiguousarray(bl),
    )


def build_program():
    nc = bacc.Bacc(get_trn_type() or "TRN2", target_bir_lowering=False, debug=False)

    F16 = mybir.dt.float16
    # x_sb[s, p, b*C2 + c] = shard[s*SUP*B + b*B + p, c] — superblocks
    # pre-arranged on host into the exact SBUF tile layout, so every DMA
    # reads 8 KB contiguous per partition.
    x_sb = nc.dram_tensor("x", [NSUP, B, SUP * C2], F16, kind="ExternalInput").ap()
    x_tail = nc.dram_tensor("xt", [B, C2], F16, kind="ExternalInput").ap()
    # all 4 weight matrices packed: w[:, k*B:(k+1)*B] = (ah, al, bh, bl)[k]
    w_in = nc.dram_tensor("w", [B, 4 * B], F16, kind="ExternalInput").ap()
    # y_raw[s, p, b*C + c] = y[s*SUP*B + b*B + p, c] — host un-permutes.
    y_out = nc.dram_tensor("y", [NSUP, B, SUP * C], F32, kind="ExternalOutput").ap()

    with ExitStack() as ctx:
        tc = ctx.enter_context(tile.TileContext(nc))
        cpool = ctx.enter_context(tc.tile_pool(name="consts", bufs=1))
        inpool = ctx.enter_context(tc.tile_pool(name="insb", bufs=6))
        outpool = ctx.enter_context(tc.tile_pool(name="outsb", bufs=3))
        pspool = ctx.enter_context(tc.tile_pool(name="ps", bufs=7, space="PSUM"))

        # One packed weight DMA on the gpsimd SWDGE queue, so it doesn't
        # wait behind the input superblocks on the sync queue.
        w_all = cpool.tile([B, 4 * B], F16, tag="w_all", name="w_all")
        nc.gpsimd.dma_start(w_all[:], w_in[:])
        w_t = {
            name: w_all[:, k * B : (k + 1) * B]
            for k, name in enumerate(("ah", "al", "bh", "bl"))
        }

        # PE HAM warmup: harmless matmuls on a zeroed tile fill the ~7 us
        # DMA head so the real matmuls start at the warm clock.
        wz = cpool.tile([B, 64], F16, tag="warmz", name="warmz")
        nc.gpsimd.memset(wz[:], 0.0)
        wps = pspool.tile([64, 64], F32, tag="warmps", name="warmps", bufs=1)
        for _ in range(40):
            nc.tensor.matmul(wps[:], wz[:], wz[:], start=True, stop=True)

        in_tiles = {}

        def load_in_sb(s):
            t = inpool.tile([B, SUP * C2], F16, tag="insb")
            if s == 0:
                # Split the first superblock into per-block DMAs so the first
                # matmuls start after 256 KB instead of 1 MB of transfer.
                for b_ in range(SUP):
                    nc.sync.dma_start(
                        t[:, b_ * C2 : (b_ + 1) * C2],
                        x_sb[0, :, b_ * C2 : (b_ + 1) * C2],
                    )
            elif s < NSUP:
                nc.sync.dma_start(t[:], x_sb[s])
            else:  # tail halo block (input block index NBLK)
                nc.sync.dma_start(t[:, 0:C2], x_tail[:])
            in_tiles[s] = t

        def rhs(sb, q, part):  # part: 0 = hi, 1 = lo
            off = q * C2 + part * C
            return in_tiles[sb][:, off : off + C]

        load_in_sb(0)
        for s in range(NSUP):
            load_in_sb(s + 1)
            out_t = outpool.tile([B, SUP * C], F32, tag="outsb")
            for q in range(SUP):
                j = s * SUP + q  # output block index; input blocks j, j+1
                cur_sb, cur_q = (j + 1) // SUP, (j + 1) % SUP
                ps = pspool.tile([B, C], F32, tag="ps")
                # y_blk = A @ x_cur + B @ x_prev at split fp16 precision:
                # M @ v ~= Mh@vh + Ml@vh + Mh@vl  (lo*lo term ~2^-24, dropped)
                # prev-block terms first: block 0 can start one DMA earlier
                mms = [
                    (w_t["bh"], rhs(s, q, 0)),
                    (w_t["bl"], rhs(s, q, 0)),
                    (w_t["bh"], rhs(s, q, 1)),
                    (w_t["ah"], rhs(cur_sb, cur_q, 0)),
                    (w_t["al"], rhs(cur_sb, cur_q, 0)),
                    (w_t["ah"], rhs(cur_sb, cur_q, 1)),
                ]
                for k, (w, r) in enumerate(mms):
                    nc.tensor.matmul(
                        ps[:], w, r, start=(k == 0), stop=(k == len(mms) - 1)
                    )
                nc.vector.tensor_copy(out_t[:, q * C : (q + 1) * C], ps[:])
            if s == NSUP - 1:
                # Per-block output DMAs for the final superblock: earlier
                # blocks ship while the last block is still computing.
                for b_ in range(SUP):
                    nc.sync.dma_start(
                        y_out[s, :, b_ * C : (b_ + 1) * C],
                        out_t[:, b_ * C : (b_ + 1) * C],
                    )
            else:
                nc.sync.dma_start(y_out[s], out_t[:])

    nc.compile()
    return nc


_prog = None


def _get_prog():
    global _prog
    if _prog is None:
        _prog = build_program()
    return _prog


def make_in_maps(signal):
    x = np.asarray(signal, dtype=np.float32)
    assert x.shape == (T, C), x.shape
    # fp16 hi/lo split, interleaved per row: [T, 2C] = [hi | lo]
    hi = x.astype(np.float16)
    lo = (x - hi.astype(np.float32)).astype(np.float16)
    xs = np.empty((T, C2), np.float16)
    xs[:, :C] = hi
    xs[:, C:] = lo
    ah, al, bh, bl = _conv_mats_fp16()
    in_maps = []
    for c in range(NCORES):
        if c == 0:
            halo = np.zeros((B, C2), np.float16)
        else:
            halo = xs[c * TL - B : c * TL]
        xc = np.concatenate([halo, xs[c * TL : (c + 1) * TL]], 0)  # [IN_ROWS, C2]
        # superblocks pre-arranged into SBUF tile layout:
        # x_sb[s, p, b*C2+c] = xc[(s*SUP + b)*B + p, c]
        x_sbm = np.ascontiguousarray(
            xc[: NSUP * SUP * B]
            .reshape(NSUP, SUP, B, C2)
            .transpose(0, 2, 1, 3)
            .reshape(NSUP, B, SUP * C2)
        )
        x_tail = np.ascontiguousarray(xc[NBLK * B :])
        w_all = np.ascontiguousarray(np.hstack([ah, al, bh, bl]))
        in_maps.append({"x": x_sbm, "xt": x_tail, "w": w_all})
    return in_maps


def unpack_y(y_raw):
    """y_raw [NSUP, B, SUP*C] -> [TL, C] (inverse of the tile layout)."""
    return np.ascontiguousarray(
        y_raw.reshape(NSUP, B, SUP, C).transpose(0, 2, 1, 3).reshape(TL, C)
    )


def run(signal, trace=False):
    """Run on the 8 NeuronCores; returns (y, BassKernelResults)."""
    nc = _get_prog()
    in_maps = make_in_maps(signal)
    last_err = None
    for _attempt in range(3):
        try:
            res = run_bass_kernel_spmd(
                nc, in_maps, core_ids=list(range(NCORES)), trace=trace
            )
            break
        except Exception as e:  # transient NRT device errors; retry
            last_err = e
    else:
        raise last_err
    y = np.concatenate(
        [unpack_y(np.asarray(res.results[c]["y"])) for c in range(NCORES)], 0
    )
    return y, res


def kernel(signal=None, **unused):
    if signal is None:
        signal = unused.pop("signal")
    y, _ = run(signal)
    return y
